# revision 10
# baseline (speedup 1.0000x reference)
"""Self-contained Trainium2 Bass kernel for 3D-RoPE multi-head attention.

Problem: x[2,2048,1020] -> qkv proj (17 heads x 60) -> 3D rotary on q,k ->
softmax attention -> out proj + bias.

Strategy: sequence-parallel across 8 NeuronCores (2 batch groups x 4 ranks,
512 rows each). Each core projects its own rows, RoPEs q/k locally, then
AllGathers rotated K and V (merged into 4 chunked collectives fired as
early as possible, in head-pair-aligned chunks) within its 4-core group,
computes attention for its local queries against the full 2048-key
sequence as each chunk lands, and projects the output rows. Matmuls run
in bf16 (f32 PSUM accumulation); softmax skips max-subtraction (logits
are ~N(0,1)); exp runs exclusively on the Scalar engine; PSUM->SBUF
copies in the projection phase run on Scalar (Vector does rope), DMAs are
batched and spread across the sync/vector/gpsimd queues.
"""

import sys

if "/opt/trn_rl_repo" not in sys.path:
    sys.path.insert(0, "/opt/trn_rl_repo")

import numpy as np
import ml_dtypes

HEADS = 17
DH = 60
D3 = 20
MIN_FREQ = 1.0 / 64.0
B, N, DIM = 2, 2048, 1020
NL = 512          # local rows per core
DIMP = 1024       # padded contraction dim (8 k-tiles)
SLOT = 64         # padded per-head column slot
NSLOT = 18        # 17 heads + 1 pad slot
MQK = NSLOT * SLOT  # 1152
NPAIR = 9         # head pairs (last pair has only head A)
KT = 8            # contraction k-tiles (1024/128)
RG = [[0, 1, 2, 3], [4, 5, 6, 7]]

# chunking: 4 collective chunks, pair-aligned
PAIRS_OF = [[0, 1], [2, 3], [4, 5], [6, 7, 8]]
CHUNK_OF_PAIR = [0, 0, 1, 1, 2, 2, 3, 3, 3]
FIRST_HEAD = [0, 4, 8, 12]
NHEADS_OF = [4, 4, 4, 5]
VCOLS_OF = [4 * 61, 4 * 61, 4 * 61, 5 * 61]   # 244,244,244,305
# k rows per pair within a chunk (pair 8 only has head A -> 64 rows)
KROWS_OF_PAIR = [128, 128, 128, 128, 128, 128, 128, 128, 64]


def _chunk_layout(j):
    """k-part offsets (elements) per pair and total sizes for chunk j."""
    offs = {}
    off = 0
    for p in PAIRS_OF[j]:
        offs[p] = off
        off += KROWS_OF_PAIR[p] * NL
    koff_end = off
    vcols = VCOLS_OF[j]
    total = koff_end + 4 * 128 * vcols
    return offs, koff_end, vcols, total


_nc_cache = {}


def _build_nc():
    from concourse import bass, tile, bacc
    import concourse.mybir as mybir
    from concourse.masks import make_identity

    BF = mybir.dt.bfloat16
    F32 = mybir.dt.float32
    AF = mybir.ActivationFunctionType
    ALU = mybir.AluOpType

    nc = bacc.Bacc("TRN2", target_bir_lowering=False, debug=False, num_devices=8)

    x_ext = nc.declare_dram_parameter("x", [NL, DIM], BF, isOutput=False)
    wqk_ext = nc.declare_dram_parameter("wqk", [2 * NPAIR, DIMP, 128], BF, isOutput=False)
    wv_exts = [
        nc.declare_dram_parameter(f"wv{j}", [DIMP, NHEADS_OF[j] * DH], BF,
                                  isOutput=False)
        for j in range(4)
    ]
    wout_ext = nc.declare_dram_parameter("wout", [MQK, DIM], BF, isOutput=False)
    cos_ext = nc.declare_dram_parameter("cos_t", [128, NL], BF, isOutput=False)
    sin_ext = nc.declare_dram_parameter("sin_t", [128, NL], BF, isOutput=False)
    perm_ext = nc.declare_dram_parameter("perm", [128, 128], BF, isOutput=False)
    out_ext = nc.declare_dram_parameter("out", [NL, DIM], F32, isOutput=True)

    SCALE = float(DH) ** -0.5

    with tile.TileContext(nc) as tc:
        with (
            tc.tile_pool(name="per", bufs=1) as per,
            tc.tile_pool(name="wrk", bufs=2) as wrk,
            tc.tile_pool(name="gat", bufs=2) as gat,
            tc.tile_pool(name="expp", bufs=4) as expp,
            tc.tile_pool(name="dram", bufs=1, space="DRAM") as dram,
        ):
            # ---------- persistent SBUF loads ----------
            cos_sb = per.tile([128, NL], BF, name="cos", tag="cos")
            nc.sync.dma_start(out=cos_sb[:], in_=cos_ext[:])
            sin_sb = per.tile([128, NL], BF, name="sin", tag="sin")
            nc.sync.dma_start(out=sin_sb[:], in_=sin_ext[:])
            perm_sb = per.tile([128, 128], BF, name="perm", tag="perm")
            nc.sync.dma_start(out=perm_sb[:], in_=perm_ext[:])

            # DRAM bounce buffers for the merged k+v collectives
            kv_loc, kv_gat = [], []
            for j in range(4):
                _, _, _, Lj = _chunk_layout(j)
                kv_loc.append(dram.tile([Lj], BF, name=f"kvl{j}", tag=f"kvl{j}"))
                kv_gat.append(dram.tile([4, Lj], BF, name=f"kvg{j}", tag=f"kvg{j}"))

            # weight prefetch (gpsimd queue): k slots in chunk order, then q
            wqkm = [None] * (2 * NPAIR)
            for m in list(range(NPAIR, 2 * NPAIR)) + list(range(NPAIR)):
                t = per.tile([128, KT * 128], BF, name=f"wqkm{m}", tag=f"wqkm{m}")
                nc.gpsimd.dma_start(
                    out=t.rearrange("p (k c) -> p k c", k=KT),
                    in_=wqk_ext[m].rearrange("(k p) c -> p k c", p=128),
                )
                wqkm[m] = t
            # v weights, one batched DMA per chunk (sync queue, after x/cos)
            wv_sb = []
            for j in range(4):
                vc = NHEADS_OF[j] * DH
                t = per.tile([128, KT, vc], BF, name=f"wvs{j}", tag=f"wvs{j}")
                nc.sync.dma_start(
                    out=t[:],
                    in_=wv_exts[j].rearrange("(k p) c -> p k c", p=128),
                )
                wv_sb.append(t)

            # ---------- phase 1: x -> xT (bf16) ----------
            ident = per.tile([128, 128], BF, name="ident", tag="ident")
            make_identity(nc, ident[:])
            xT_sb = []
            for k in range(KT):
                t = per.tile([128, NL], BF, name=f"xT{k}", tag=f"xT{k}")
                xT_sb.append(t)
            nc.vector.memset(xT_sb[KT - 1][:], 0.0)

            rotq_sb = [
                per.tile([128, NL], BF, name=f"rotq{m}", tag=f"rotq{m}")
                for m in range(NPAIR)
            ]

            with tc.tile_pool(name="psP", bufs=2, space="PSUM") as psP:
                for mt in range(4):
                    xt = wrk.tile([128, DIM], BF, name="xrow", tag="xrow")
                    nc.sync.dma_start(out=xt[:], in_=x_ext[mt * 128:(mt + 1) * 128, :])
                    for k in range(KT):
                        kk = min(128, DIM - k * 128)  # 124 on last tile
                        pt = psP.tile([128, 128], BF, name="tp", tag="tp")
                        nc.tensor.transpose(
                            pt[0:kk, :], xt[:, k * 128:k * 128 + kk], ident[:]
                        )
                        nc.vector.tensor_copy(
                            xT_sb[k][0:kk, mt * 128:(mt + 1) * 128], pt[0:kk, :]
                        )

                def qk_tile(m, dest):
                    # m: M-tile index into [0, 18): 0-8 q slots, 9-17 k slots
                    pqk = psP.tile([128, NL], F32, name="qk", tag="qk", bufs=3)
                    for k in range(KT):
                        nc.tensor.matmul(
                            pqk[:],
                            lhsT=wqkm[m][:, k * 128:(k + 1) * 128],
                            rhs=xT_sb[k][:],
                            start=(k == 0),
                            stop=(k == KT - 1),
                        )
                    qkbf = wrk.tile([128, NL], BF, name="qkbf", tag="qkbf")
                    nc.scalar.activation(qkbf[:], pqk[:], AF.Copy)
                    psw = psP.tile([128, NL], F32, name="sw", tag="sw")
                    nc.tensor.matmul(psw[:], lhsT=perm_sb[:], rhs=qkbf[:])
                    swbf = wrk.tile([128, NL], BF, name="swbf", tag="swbf")
                    nc.scalar.activation(swbf[:], psw[:], AF.Copy)
                    t1 = wrk.tile([128, NL], BF, name="t1", tag="t1")
                    nc.vector.tensor_tensor(t1[:], qkbf[:], cos_sb[:], ALU.mult)
                    t2 = wrk.tile([128, NL], BF, name="t2", tag="t2")
                    nc.vector.tensor_tensor(t2[:], swbf[:], sin_sb[:], ALU.mult)
                    nc.vector.tensor_tensor(dest[:], t1[:], t2[:], ALU.add)

                def k_pair(p):
                    j = CHUNK_OF_PAIR[p]
                    offs, _, _, _ = _chunk_layout(j)
                    rows = KROWS_OF_PAIR[p]
                    dest = wrk.tile([128, NL], BF, name="rotk", tag="rotk")
                    qk_tile(NPAIR + p, dest)
                    nc.sync.dma_start(
                        out=kv_loc[j][offs[p]:offs[p] + rows * NL]
                        .rearrange("(p n) -> p n", p=rows),
                        in_=dest[0:rows, :],
                    )

                def v_chunk(j):
                    offs, koff_end, vcols, _ = _chunk_layout(j)
                    nh = NHEADS_OF[j]
                    nc_cols = nh * DH
                    for mt in range(4):
                        vxt = wrk.tile([128, 305], BF, name="vx", tag="vx")
                        ones_ap = vxt[:, 0:vcols].rearrange(
                            "p (h c) -> p h c", c=61
                        )[:, :, 0:1]
                        nc.gpsimd.memset(ones_ap, 1.0)
                        pv = psP.tile([128, 384], F32, name="pv", tag="qk", bufs=3)
                        for k in range(KT):
                            nc.tensor.matmul(
                                pv[:, 0:nc_cols],
                                lhsT=xT_sb[k][:, mt * 128:(mt + 1) * 128],
                                rhs=wv_sb[j][:, k, :],
                                start=(k == 0),
                                stop=(k == KT - 1),
                            )
                        src = pv[:, 0:nc_cols].rearrange("p (h d) -> p h d", d=DH)
                        dst = vxt[:, 0:vcols].rearrange(
                            "p (h c) -> p h c", c=61
                        )[:, :, 1:61]
                        nc.vector.tensor_copy(dst, src)
                        nc.sync.dma_start(
                            out=kv_loc[j][koff_end + mt * 128 * vcols:
                                          koff_end + (mt + 1) * 128 * vcols]
                            .rearrange("(p c) -> p c", p=128),
                            in_=vxt[:, 0:vcols],
                        )

                def ag_fire(j):
                    nc.gpsimd.collective_compute(
                        "AllGather", ALU.bypass,
                        ins=[kv_loc[j].opt()], outs=[kv_gat[j].opt()],
                        replica_groups=RG,
                    )

                # k+v projections in chunk order, firing AGs as ready
                for j in range(4):
                    for p in PAIRS_OF[j]:
                        k_pair(p)
                    v_chunk(j)
                    ag_fire(j)

                # q projections (needed once attention starts)
                for m in range(NPAIR):
                    qk_tile(m, rotq_sb[m])

            # wout loads (batched, gpsimd queue; needed only at the tail)
            wout_sb = []
            for g in range(3):
                t = per.tile([128, 3, DIM], BF, name=f"wout{g}", tag=f"wout{g}")
                nc.gpsimd.dma_start(
                    out=t[:],
                    in_=wout_ext[g * 384:(g + 1) * 384, :]
                    .rearrange("(t p) c -> p t c", p=128),
                )
                wout_sb.append(t)

            # ---------- attention ----------
            aoT = [
                per.tile([128, NL], BF, name=f"aoT{p}", tag=f"aoT{p}")
                for p in range(NPAIR)
            ]
            for p in range(NPAIR):
                nc.gpsimd.memset(aoT[p][:], 0.0)
            # bias row (inner index 1088 = slot 17 row 0 -> tile 8, partition 64)
            nc.vector.memset(aoT[NPAIR - 1][64:65, :], 1.0)

            # gathered tiles (SBUF), batched DMAs per chunk; the ktp/vxg
            # tiles rotate through 2 buffers (chunk j+2 reuses chunk j's)
            ktp_sb = []    # [128, 2, 4, NL]: (slot-row, pair-in-chunk, rank, keypos)
            vxg_sb = []    # [128, 4, 4, vcols]: (keypos-in-mt, rank, mt, vcol)
            ktp8_sb = None
            for j in range(4):
                offs, koff_end, vcols, Lj = _chunk_layout(j)
                kt_t = gat.tile([128, 2, 4, NL], BF, name=f"ktp{j}", tag="ktp")
                for r in range(4):
                    nc.sync.dma_start(
                        out=kt_t[:, :, r, :],
                        in_=kv_gat[j][r, 0:2 * 128 * NL]
                        .rearrange("(pp p n) -> p pp n", pp=2, p=128),
                    )
                ktp_sb.append(kt_t)
                if j == 3:
                    ktp8_sb = per.tile([64, 4, NL], BF, name="ktp8", tag="ktp8")
                    nc.sync.dma_start(
                        out=ktp8_sb[:],
                        in_=kv_gat[3][:, 2 * 128 * NL:2 * 128 * NL + 64 * NL]
                        .rearrange("r (p n) -> p r n", p=64),
                    )
                vx_t = gat.tile([128, 4, 4, 305], BF, name=f"vxg{j}", tag="vxg")
                for r in range(4):
                    nc.sync.dma_start(
                        out=vx_t[:, r, :, 0:vcols],
                        in_=kv_gat[j][r, koff_end:koff_end + 4 * 128 * vcols]
                        .rearrange("(mt p c) -> p mt c", mt=4, p=128),
                    )
                vxg_sb.append(vx_t)

            with (
                tc.tile_pool(name="psD", bufs=2, space="PSUM") as psD,
                tc.tile_pool(name="psAV", bufs=4, space="PSUM") as psAV,
            ):
                def pair_attention(p):
                    j = CHUNK_OF_PAIR[p]
                    two = p < NPAIR - 1
                    pp = PAIRS_OF[j].index(p)
                    lA = (2 * p - FIRST_HEAD[j]) * 61
                    lB = lA + 61
                    av = psAV.tile([128, NL], F32, name="av", tag="av")
                    for c in range(16):
                        r, i = c // 4, c % 4
                        if p == NPAIR - 1:
                            kA = ktp8_sb[0:DH, r, i * 128:(i + 1) * 128]
                            kB = None
                        else:
                            kA = ktp_sb[j][0:DH, pp, r, i * 128:(i + 1) * 128]
                            kB = ktp_sb[j][64:64 + DH, pp, r, i * 128:(i + 1) * 128]
                        dots = psD.tile([128, 1024], F32, name="dots", tag="dots")
                        nc.tensor.matmul(dots[:, 0:NL], lhsT=kA,
                                         rhs=rotq_sb[p][0:DH, :])
                        if two:
                            nc.tensor.matmul(dots[:, NL:2 * NL], lhsT=kB,
                                             rhs=rotq_sb[p][64:64 + DH, :])
                        et = expp.tile([128, 1024], BF, name="et", tag="expT")
                        width = 1024 if two else NL
                        nc.scalar.activation(et[:, 0:width], dots[:, 0:width],
                                             AF.Exp, scale=SCALE)
                        nc.tensor.matmul(
                            av[0:61, :],
                            lhsT=vxg_sb[j][:, r, i, lA:lA + 61],
                            rhs=et[:, 0:NL],
                            start=(c == 0), stop=(c == 15),
                        )
                        if two:
                            nc.tensor.matmul(
                                av[64:125, :],
                                lhsT=vxg_sb[j][:, r, i, lB:lB + 61],
                                rhs=et[:, NL:2 * NL],
                                start=(c == 0), stop=(c == 15),
                            )
                    # epilogue: divide by the ones-row denominator
                    rcA = wrk.tile([1, NL], F32, name="rc", tag="rc")
                    rcA_s = wrk.tile([1, NL], F32, name="rcs", tag="rcs")
                    nc.vector.tensor_copy(rcA_s[:], av[0:1, :])
                    nc.vector.reciprocal_approx_fast(rcA[:], rcA_s[:])
                    bc = wrk.tile([128, NL], F32, name="bc", tag="bc")
                    nc.gpsimd.partition_broadcast(bc[0:61, :], rcA[:])
                    nc.vector.tensor_tensor(
                        aoT[p][0:61, :], av[0:61, :], bc[0:61, :], ALU.mult
                    )
                    if two:
                        rcB = wrk.tile([1, NL], F32, name="rc", tag="rc")
                        rcB_s = wrk.tile([1, NL], F32, name="rcs", tag="rcs")
                        nc.vector.tensor_copy(rcB_s[:], av[64:65, :])
                        nc.vector.reciprocal_approx_fast(rcB[:], rcB_s[:])
                        bc2 = wrk.tile([128, NL], F32, name="bc2", tag="bc2")
                        nc.gpsimd.partition_broadcast(bc2[0:61, :], rcB[:])
                        nc.vector.tensor_tensor(
                            aoT[p][64:125, :], av[64:125, :], bc2[0:61, :],
                            ALU.mult,
                        )

                for p in range(NPAIR):
                    pair_attention(p)

            # ---------- output projection ----------
            with tc.tile_pool(name="psO", bufs=2, space="PSUM") as psO:
                for mt in range(4):
                    for (n0, n1) in ((0, 510), (510, 1020)):
                        po = psO.tile([128, 510], F32, name="po", tag="o")
                        for kt in range(NPAIR):
                            nc.tensor.matmul(
                                po[:],
                                lhsT=aoT[kt][:, mt * 128:(mt + 1) * 128],
                                rhs=wout_sb[kt // 3][:, kt % 3, n0:n1],
                                start=(kt == 0),
                                stop=(kt == NPAIR - 1),
                            )
                        ot = wrk.tile([128, 510], F32, name="ot", tag="ot")
                        nc.vector.tensor_copy(ot[:], po[:])
                        nc.sync.dma_start(
                            out=out_ext[mt * 128:(mt + 1) * 128, n0:n1], in_=ot[:]
                        )

    nc.finalize()
    return nc


def _host_prep(x, coords, w_qkv, w_out, b_out):
    bf16 = ml_dtypes.bfloat16
    x = np.asarray(x, np.float32)
    coords = np.asarray(coords, np.float32)
    w_qkv = np.asarray(w_qkv, np.float32)
    w_out = np.asarray(w_out, np.float32)
    b_out = np.asarray(b_out, np.float32)

    # weights: q/k into 64-wide head slots, [1024, 2*1152]
    wqk = np.zeros((DIMP, 2 * MQK), np.float32)
    wq = w_qkv[:, 0:DIM].reshape(DIM, HEADS, DH)
    wk = w_qkv[:, DIM:2 * DIM].reshape(DIM, HEADS, DH)
    t = np.zeros((DIM, NSLOT, SLOT), np.float32)
    t[:, :HEADS, :DH] = wq
    wqk[:DIM, 0:MQK] = t.reshape(DIM, MQK)
    t[:] = 0.0
    t[:, :HEADS, :DH] = wk
    wqk[:DIM, MQK:2 * MQK] = t.reshape(DIM, MQK)
    # M-tile-major: [18, 1024, 128]
    wqk = np.ascontiguousarray(
        wqk.reshape(DIMP, 2 * NPAIR, 128).transpose(1, 0, 2)
    ).astype(bf16)

    # v weights, per-chunk contiguous [1024, nh*60]
    wv_full = np.zeros((DIMP, DIM), np.float32)
    wv_full[:DIM, :] = w_qkv[:, 2 * DIM:3 * DIM]
    wv_chunks = []
    for j in range(4):
        fh, nh = FIRST_HEAD[j], NHEADS_OF[j]
        wv_chunks.append(
            np.ascontiguousarray(wv_full[:, fh * DH:(fh + nh) * DH]).astype(bf16)
        )

    wout = np.zeros((NSLOT, SLOT, DIM), np.float32)
    wout[:HEADS, 1:DH + 1, :] = w_out.reshape(HEADS, DH, DIM)
    wout[NSLOT - 1, 0, :] = b_out  # bias row at inner index 1088
    wout = wout.reshape(MQK, DIM).astype(bf16)

    # permutation matrix: out[m] = q[partner(m)] (rotate-half pair swap)
    perm = np.zeros((128, 128), np.float32)
    for m in range(128):
        a = m % SLOT
        if a < DH:
            pos = a % D3
            partner = (m // SLOT) * SLOT + (a // D3) * D3 + (
                pos + 10 if pos < 10 else pos - 10
            )
            perm[partner, m] = 1.0
    perm = perm.astype(bf16)

    # rotary tables per core: [128, 512] two identical 64-row head slots
    inv_freq = 1.0 / (10000.0 ** (np.arange(0, D3, 2, dtype=np.float32) / D3))  # [10]
    j = np.arange(SLOT)
    axis_of = np.clip(j // D3, 0, 2)
    jj = (j % D3) % 10
    sign = np.where((j % D3) < 10, -1.0, 1.0).astype(np.float32)
    valid = (j < DH).astype(np.float32)

    in_maps = []
    outs_meta = []
    for c in range(8):
        g, r = c // 4, c % 4
        rows = slice(r * NL, (r + 1) * NL)
        x_loc = np.ascontiguousarray(x[g, rows, :]).astype(bf16)
        t_axis = coords[g, rows, :]  # [NL, 3]
        f = (t_axis[:, axis_of] / MIN_FREQ) * inv_freq[jj][None, :]  # [NL, 64]
        cos_t = (np.cos(f) * valid[None, :]).T.astype(np.float32)  # [64, NL]
        sin_t = (np.sin(f) * (sign * valid)[None, :]).T.astype(np.float32)
        cos_full = np.concatenate([cos_t, cos_t], axis=0).astype(bf16)  # [128, NL]
        sin_full = np.concatenate([sin_t, sin_t], axis=0).astype(bf16)
        im = {
            "x": x_loc,
            "wqk": wqk,
            "wout": wout,
            "cos_t": cos_full,
            "sin_t": sin_full,
            "perm": perm,
        }
        for jj2 in range(4):
            im[f"wv{jj2}"] = wv_chunks[jj2]
        in_maps.append(im)
        outs_meta.append((g, rows))
    return in_maps, outs_meta


def kernel(x, coords, w_qkv, w_out, b_out, _trace=False, _dbg=None):
    from concourse import bass_utils

    in_maps, outs_meta = _host_prep(x, coords, w_qkv, w_out, b_out)
    if "nc" not in _nc_cache:
        _nc_cache["nc"] = _build_nc()
    nc = _nc_cache["nc"]
    last_err = None
    for _attempt in range(3):
        try:
            res = bass_utils.run_bass_kernel_spmd(
                nc, in_maps, core_ids=list(range(8)), trace=_trace
            )
            break
        except Exception as e:  # transient axon worker failures
            last_err = e
            import time as _time
            _time.sleep(2.0)
    else:
        raise last_err
    out = np.empty((B, N, DIM), np.float32)
    for c, (g, rows) in enumerate(outs_meta):
        out[g, rows, :] = res.results[c]["out"]
    if _trace:
        kernel.last_exec_time_ns = res.exec_time_ns
    return out


# revision 18
# speedup vs baseline: 1.1299x; 1.1299x over previous
"""Self-contained Trainium2 Bass kernel for 3D-RoPE multi-head attention.

Problem: x[2,2048,1020] -> qkv proj (17 heads x 60) -> 3D rotary on q,k ->
softmax attention -> out proj + bias.

Strategy: sequence-parallel across 8 NeuronCores (2 batch groups x 4 ranks,
512 rows each). Each core projects its own rows, RoPEs q/k locally, then
AllGathers rotated K and V (merged into 4 chunked collectives fired as
early as possible, in head-pair-aligned chunks) within its 4-core group,
computes attention for its local queries against the full 2048-key
sequence as each chunk lands, and projects the output rows. Matmuls run
in bf16 (f32 PSUM accumulation); softmax skips max-subtraction (logits
are ~N(0,1)); exp runs exclusively on the Scalar engine; PSUM->SBUF
copies in the projection phase run on Scalar (Vector does rope), DMAs are
batched and spread across the sync/vector/gpsimd queues.
"""

import sys

if "/opt/trn_rl_repo" not in sys.path:
    sys.path.insert(0, "/opt/trn_rl_repo")

import numpy as np
import ml_dtypes

HEADS = 17
DH = 60
D3 = 20
MIN_FREQ = 1.0 / 64.0
B, N, DIM = 2, 2048, 1020
NL = 512          # local rows per core
DIMP = 1024       # padded contraction dim (8 k-tiles)
SLOT = 64         # padded per-head column slot
NSLOT = 18        # 17 heads + 1 pad slot
MQK = NSLOT * SLOT  # 1152
NPAIR = 9         # head pairs (last pair has only head A)
KT = 8            # contraction k-tiles (1024/128)
RG = [[0, 1, 2, 3], [4, 5, 6, 7]]

# chunking: 4 collective chunks, pair-aligned
PAIRS_OF = [[0], [1, 2], [3, 4, 5], [6, 7, 8]]
CHUNK_OF_PAIR = [0, 1, 1, 2, 2, 2, 3, 3, 3]
FIRST_HEAD = [0, 2, 6, 12]
NHEADS_OF = [2, 4, 6, 5]
VCOLS_OF = [2 * 61, 4 * 61, 6 * 61, 5 * 61]   # 122,244,366,305
# k rows per pair within a chunk (pair 8 only has head A -> 64 rows)
KROWS_OF_PAIR = [128, 128, 128, 128, 128, 128, 128, 128, 64]


def _chunk_layout(j):
    """k-part offsets (elements) per pair and total sizes for chunk j."""
    offs = {}
    off = 0
    for p in PAIRS_OF[j]:
        offs[p] = off
        off += KROWS_OF_PAIR[p] * NL
    koff_end = off
    vcols = VCOLS_OF[j]
    total = koff_end + 4 * 128 * vcols
    return offs, koff_end, vcols, total


_nc_cache = {}


def _build_nc():
    from concourse import bass, tile, bacc
    import concourse.mybir as mybir
    from concourse.masks import make_identity

    BF = mybir.dt.bfloat16
    F32 = mybir.dt.float32
    AF = mybir.ActivationFunctionType
    ALU = mybir.AluOpType

    nc = bacc.Bacc("TRN2", target_bir_lowering=False, debug=False, num_devices=8)

    # weight tensors are host-prepped in partition-major layout so every
    # DMA line is fully contiguous
    x_ext = nc.declare_dram_parameter("x", [NL, DIM], BF, isOutput=False)
    wqk_ext = nc.declare_dram_parameter("wqk", [2 * NPAIR, 128, KT * 128], BF,
                                        isOutput=False)
    wv_exts = [
        nc.declare_dram_parameter(f"wv{j}", [128, KT * NHEADS_OF[j] * DH], BF,
                                  isOutput=False)
        for j in range(4)
    ]
    wout_ext = nc.declare_dram_parameter("wout", [3, 128, 3 * DIM], BF,
                                         isOutput=False)
    cos_ext = nc.declare_dram_parameter("cos_t", [128, NL], BF, isOutput=False)
    sin_ext = nc.declare_dram_parameter("sin_t", [128, NL], BF, isOutput=False)
    perm_ext = nc.declare_dram_parameter("perm", [128, 128], BF, isOutput=False)
    out_ext = nc.declare_dram_parameter("out", [NL, DIM], F32, isOutput=True)

    SCALE = float(DH) ** -0.5

    with tile.TileContext(nc) as tc:
        with (
            tc.tile_pool(name="per", bufs=1) as per,
            tc.tile_pool(name="wrk", bufs=2) as wrk,
            tc.tile_pool(name="gat", bufs=2) as gat,
            tc.tile_pool(name="expp", bufs=4) as expp,
            tc.tile_pool(name="dram", bufs=1, space="DRAM") as dram,
        ):
            # ---------- persistent SBUF loads (x rows first!) ----------
            x_sb = []
            for mt in range(4):
                xt = per.tile([128, DIM], BF, name=f"xrow{mt}", tag=f"xrow{mt}")
                nc.sync.dma_start(out=xt[:], in_=x_ext[mt * 128:(mt + 1) * 128, :])
                x_sb.append(xt)
            cos_sb = per.tile([128, NL], BF, name="cos", tag="cos")
            nc.sync.dma_start(out=cos_sb[:], in_=cos_ext[:])
            sin_sb = per.tile([128, NL], BF, name="sin", tag="sin")
            nc.sync.dma_start(out=sin_sb[:], in_=sin_ext[:])
            perm_sb = per.tile([128, 128], BF, name="perm", tag="perm")
            nc.sync.dma_start(out=perm_sb[:], in_=perm_ext[:])

            # DRAM bounce buffers for the merged k+v collectives
            kv_loc, kv_gat = [], []
            for j in range(4):
                _, _, _, Lj = _chunk_layout(j)
                kv_loc.append(dram.tile([Lj], BF, name=f"kvl{j}", tag=f"kvl{j}"))
                kv_gat.append(dram.tile([4, Lj], BF, name=f"kvg{j}", tag=f"kvg{j}"))

            # weight prefetch (gpsimd queue): k slots in chunk order, then q
            wqkm = [None] * (2 * NPAIR)
            for m in list(range(NPAIR, 2 * NPAIR)) + list(range(NPAIR)):
                t = per.tile([128, KT * 128], BF, name=f"wqkm{m}", tag=f"wqkm{m}")
                nc.gpsimd.dma_start(out=t[:], in_=wqk_ext[m])
                wqkm[m] = t
            # v weights, one contiguous DMA per chunk (sync queue, after x)
            wv_sb = []
            for j in range(4):
                vc = NHEADS_OF[j] * DH
                t = per.tile([128, KT, vc], BF, name=f"wvs{j}", tag=f"wvs{j}")
                nc.sync.dma_start(
                    out=t[:],
                    in_=wv_exts[j].rearrange("p (k c) -> p k c", k=KT),
                )
                wv_sb.append(t)

            # ---------- phase 1: x -> xT (bf16) ----------
            ident = per.tile([128, 128], BF, name="ident", tag="ident")
            make_identity(nc, ident[:])
            xT_sb = []
            for k in range(KT):
                t = per.tile([128, NL], BF, name=f"xT{k}", tag=f"xT{k}")
                xT_sb.append(t)
            nc.vector.memset(xT_sb[KT - 1][:], 0.0)

            rotq_sb = [
                per.tile([128, NL], BF, name=f"rotq{m}", tag=f"rotq{m}")
                for m in range(NPAIR)
            ]

            with tc.tile_pool(name="psP", bufs=2, space="PSUM") as psP:
                for mt in range(4):
                    for k in range(KT):
                        kk = min(128, DIM - k * 128)  # 124 on last tile
                        pt = psP.tile([128, 128], BF, name="tp", tag="tp")
                        nc.tensor.transpose(
                            pt[0:kk, :], x_sb[mt][:, k * 128:k * 128 + kk], ident[:]
                        )
                        nc.vector.tensor_copy(
                            xT_sb[k][0:kk, mt * 128:(mt + 1) * 128], pt[0:kk, :]
                        )

                def qk_tile(m, dest):
                    # m: M-tile index into [0, 18): 0-8 q slots, 9-17 k slots
                    pqk = psP.tile([128, NL], F32, name="qk", tag="qk", bufs=3)
                    for k in range(KT):
                        nc.tensor.matmul(
                            pqk[:],
                            lhsT=wqkm[m][:, k * 128:(k + 1) * 128],
                            rhs=xT_sb[k][:],
                            start=(k == 0),
                            stop=(k == KT - 1),
                        )
                    qkbf = wrk.tile([128, NL], BF, name="qkbf", tag="qkbf")
                    nc.vector.tensor_copy(qkbf[:], pqk[:])
                    psw = psP.tile([128, NL], F32, name="sw", tag="sw")
                    nc.tensor.matmul(psw[:], lhsT=perm_sb[:], rhs=qkbf[:])
                    t1 = wrk.tile([128, NL], BF, name="t1", tag="t1")
                    nc.vector.tensor_tensor(t1[:], qkbf[:], cos_sb[:], ALU.mult)
                    t2 = wrk.tile([128, NL], BF, name="t2", tag="t2")
                    nc.vector.tensor_tensor(t2[:], psw[:], sin_sb[:], ALU.mult)
                    nc.vector.tensor_tensor(dest[:], t1[:], t2[:], ALU.add)

                def k_pair(p):
                    j = CHUNK_OF_PAIR[p]
                    offs, _, _, _ = _chunk_layout(j)
                    rows = KROWS_OF_PAIR[p]
                    dest = wrk.tile([128, NL], BF, name="rotk", tag="rotk")
                    qk_tile(NPAIR + p, dest)
                    nc.sync.dma_start(
                        out=kv_loc[j][offs[p]:offs[p] + rows * NL]
                        .rearrange("(p n) -> p n", p=rows),
                        in_=dest[0:rows, :],
                    )

                def v_chunk(j):
                    offs, koff_end, vcols, _ = _chunk_layout(j)
                    nh = NHEADS_OF[j]
                    nc_cols = nh * DH
                    for mt in range(4):
                        vxt = wrk.tile([128, 366], BF, name="vx", tag="vx")
                        ones_ap = vxt[:, 0:vcols].rearrange(
                            "p (h c) -> p h c", c=61
                        )[:, :, 0:1]
                        nc.gpsimd.memset(ones_ap, 1.0)
                        pv = psP.tile([128, 384], F32, name="pv", tag="qk", bufs=3)
                        for k in range(KT):
                            nc.tensor.matmul(
                                pv[:, 0:nc_cols],
                                lhsT=xT_sb[k][:, mt * 128:(mt + 1) * 128],
                                rhs=wv_sb[j][:, k, :],
                                start=(k == 0),
                                stop=(k == KT - 1),
                            )
                        src = pv[:, 0:nc_cols].rearrange("p (h d) -> p h d", d=DH)
                        dst = vxt[:, 0:vcols].rearrange(
                            "p (h c) -> p h c", c=61
                        )[:, :, 1:61]
                        nc.vector.tensor_copy(dst, src)
                        nc.sync.dma_start(
                            out=kv_loc[j][koff_end + mt * 128 * vcols:
                                          koff_end + (mt + 1) * 128 * vcols]
                            .rearrange("(p c) -> p c", p=128),
                            in_=vxt[:, 0:vcols],
                        )

                def ag_fire(j):
                    nc.gpsimd.collective_compute(
                        "AllGather", ALU.bypass,
                        ins=[kv_loc[j].opt()], outs=[kv_gat[j].opt()],
                        replica_groups=RG,
                    )

                # k+v projections in chunk order, firing AGs as ready;
                # q projections interleaved between chunks so early pairs'
                # rotq are ready as soon as their AG lands
                Q_AFTER = [[0], [1, 2], [3, 4, 5], [6, 7, 8]]
                for j in range(4):
                    for p in PAIRS_OF[j]:
                        k_pair(p)
                    v_chunk(j)
                    ag_fire(j)
                    for m in Q_AFTER[j]:
                        qk_tile(m, rotq_sb[m])

            # wout loads (batched, gpsimd queue; needed only at the tail)
            wout_sb = []
            for g in range(3):
                t = per.tile([128, 3, DIM], BF, name=f"wout{g}", tag=f"wout{g}")
                nc.gpsimd.dma_start(
                    out=t[:],
                    in_=wout_ext[g].rearrange("p (t c) -> p t c", t=3),
                )
                wout_sb.append(t)

            # ---------- attention ----------
            aoT = [
                per.tile([128, NL], BF, name=f"aoT{p}", tag=f"aoT{p}")
                for p in range(NPAIR)
            ]
            for p in range(NPAIR):
                nc.gpsimd.memset(aoT[p][:], 0.0)
            # bias row (inner index 1088 = slot 17 row 0 -> tile 8, partition 64)
            nc.vector.memset(aoT[NPAIR - 1][64:65, :], 1.0)

            # gathered tiles (SBUF), batched DMAs per chunk; the ktp/vxg
            # tiles rotate through 2 buffers (chunk j+2 reuses chunk j's)
            ktp_sb = []    # [128, 2, 4, NL]: (slot-row, pair-in-chunk, rank, keypos)
            vxg_sb = []    # [128, 4, 4, vcols]: (keypos-in-mt, rank, mt, vcol)
            ktp8_sb = None
            for j in range(4):
                offs, koff_end, vcols, Lj = _chunk_layout(j)
                n128 = len([p for p in PAIRS_OF[j] if KROWS_OF_PAIR[p] == 128])
                kt_t = gat.tile([128, 3, 4, NL], BF, name=f"ktp{j}", tag="ktp")
                for r in range(4):
                    nc.sync.dma_start(
                        out=kt_t[:, 0:n128, r, :],
                        in_=kv_gat[j][r, 0:n128 * 128 * NL]
                        .rearrange("(pp p n) -> p pp n", pp=n128, p=128),
                    )
                ktp_sb.append(kt_t)
                if j == 3:
                    ktp8_sb = per.tile([64, 4, NL], BF, name="ktp8", tag="ktp8")
                    nc.sync.dma_start(
                        out=ktp8_sb[:],
                        in_=kv_gat[3][:, n128 * 128 * NL:n128 * 128 * NL + 64 * NL]
                        .rearrange("r (p n) -> p r n", p=64),
                    )
                vx_t = gat.tile([128, 4, 4, 366], BF, name=f"vxg{j}", tag="vxg")
                for r in range(4):
                    nc.sync.dma_start(
                        out=vx_t[:, r, :, 0:vcols],
                        in_=kv_gat[j][r, koff_end:koff_end + 4 * 128 * vcols]
                        .rearrange("(mt p c) -> p mt c", mt=4, p=128),
                    )
                vxg_sb.append(vx_t)

            with (
                tc.tile_pool(name="psD", bufs=2, space="PSUM") as psD,
                tc.tile_pool(name="psAV", bufs=4, space="PSUM") as psAV,
            ):
                def pair_attention(p):
                    j = CHUNK_OF_PAIR[p]
                    two = p < NPAIR - 1
                    pp = PAIRS_OF[j].index(p)
                    lA = (2 * p - FIRST_HEAD[j]) * 61
                    lB = lA + 61
                    av = psAV.tile([128, NL], F32, name="av", tag="av")
                    for c in range(16):
                        r, i = c // 4, c % 4
                        if p == NPAIR - 1:
                            kA = ktp8_sb[0:DH, r, i * 128:(i + 1) * 128]
                            kB = None
                        else:
                            kA = ktp_sb[j][0:DH, pp, r, i * 128:(i + 1) * 128]
                            kB = ktp_sb[j][64:64 + DH, pp, r, i * 128:(i + 1) * 128]
                        dots = psD.tile([128, 1024], F32, name="dots", tag="dots")
                        nc.tensor.matmul(dots[:, 0:NL], lhsT=kA,
                                         rhs=rotq_sb[p][0:DH, :])
                        if two:
                            nc.tensor.matmul(dots[:, NL:2 * NL], lhsT=kB,
                                             rhs=rotq_sb[p][64:64 + DH, :])
                        et = expp.tile([128, 1024], BF, name="et", tag="expT")
                        width = 1024 if two else NL
                        nc.scalar.activation(et[:, 0:width], dots[:, 0:width],
                                             AF.Exp, scale=SCALE)
                        nc.tensor.matmul(
                            av[0:61, :],
                            lhsT=vxg_sb[j][:, r, i, lA:lA + 61],
                            rhs=et[:, 0:NL],
                            start=(c == 0), stop=(c == 15),
                        )
                        if two:
                            nc.tensor.matmul(
                                av[64:125, :],
                                lhsT=vxg_sb[j][:, r, i, lB:lB + 61],
                                rhs=et[:, NL:2 * NL],
                                start=(c == 0), stop=(c == 15),
                            )
                    # epilogue: divide by the ones-row denominator
                    rcA = wrk.tile([1, NL], F32, name="rc", tag="rc")
                    rcA_s = wrk.tile([1, NL], F32, name="rcs", tag="rcs")
                    nc.vector.tensor_copy(rcA_s[:], av[0:1, :])
                    nc.vector.reciprocal_approx_fast(rcA[:], rcA_s[:])
                    bc = wrk.tile([128, NL], F32, name="bc", tag="bc")
                    nc.gpsimd.partition_broadcast(bc[0:61, :], rcA[:])
                    nc.vector.tensor_tensor(
                        aoT[p][0:61, :], av[0:61, :], bc[0:61, :], ALU.mult
                    )
                    if two:
                        rcB = wrk.tile([1, NL], F32, name="rc", tag="rc")
                        rcB_s = wrk.tile([1, NL], F32, name="rcs", tag="rcs")
                        nc.vector.tensor_copy(rcB_s[:], av[64:65, :])
                        nc.vector.reciprocal_approx_fast(rcB[:], rcB_s[:])
                        bc2 = wrk.tile([128, NL], F32, name="bc2", tag="bc2")
                        nc.gpsimd.partition_broadcast(bc2[0:61, :], rcB[:])
                        nc.vector.tensor_tensor(
                            aoT[p][64:125, :], av[64:125, :], bc2[0:61, :],
                            ALU.mult,
                        )

                for p in range(NPAIR):
                    pair_attention(p)

            # ---------- output projection ----------
            with tc.tile_pool(name="psO", bufs=2, space="PSUM") as psO:
                for mt in range(4):
                    for (n0, n1) in ((0, 510), (510, 1020)):
                        po = psO.tile([128, 510], F32, name="po", tag="o")
                        for kt in range(NPAIR):
                            nc.tensor.matmul(
                                po[:],
                                lhsT=aoT[kt][:, mt * 128:(mt + 1) * 128],
                                rhs=wout_sb[kt // 3][:, kt % 3, n0:n1],
                                start=(kt == 0),
                                stop=(kt == NPAIR - 1),
                            )
                        ot = wrk.tile([128, 510], F32, name="ot", tag="ot")
                        nc.vector.tensor_copy(ot[:], po[:])
                        nc.sync.dma_start(
                            out=out_ext[mt * 128:(mt + 1) * 128, n0:n1], in_=ot[:]
                        )

    nc.finalize()
    return nc


def _host_prep(x, coords, w_qkv, w_out, b_out):
    bf16 = ml_dtypes.bfloat16
    x = np.asarray(x, np.float32)
    coords = np.asarray(coords, np.float32)
    w_qkv = np.asarray(w_qkv, np.float32)
    w_out = np.asarray(w_out, np.float32)
    b_out = np.asarray(b_out, np.float32)

    # weights: q/k into 64-wide head slots, [1024, 2*1152]
    wqk = np.zeros((DIMP, 2 * MQK), np.float32)
    wq = w_qkv[:, 0:DIM].reshape(DIM, HEADS, DH)
    wk = w_qkv[:, DIM:2 * DIM].reshape(DIM, HEADS, DH)
    t = np.zeros((DIM, NSLOT, SLOT), np.float32)
    t[:, :HEADS, :DH] = wq
    wqk[:DIM, 0:MQK] = t.reshape(DIM, MQK)
    t[:] = 0.0
    t[:, :HEADS, :DH] = wk
    wqk[:DIM, MQK:2 * MQK] = t.reshape(DIM, MQK)
    # M-tile-major then partition-major: [18, 128, KT*128]
    # arr[m, p, k*128+c] = wqk[k*128+p, m*128+c]
    wqk = np.ascontiguousarray(
        wqk.reshape(KT, 128, 2 * NPAIR, 128).transpose(2, 1, 0, 3)
        .reshape(2 * NPAIR, 128, KT * 128)
    ).astype(bf16)

    # v weights, per-chunk partition-major [128, KT*nh*60]
    wv_full = np.zeros((DIMP, DIM), np.float32)
    wv_full[:DIM, :] = w_qkv[:, 2 * DIM:3 * DIM]
    wv_chunks = []
    for j in range(4):
        fh, nh = FIRST_HEAD[j], NHEADS_OF[j]
        blk = wv_full[:, fh * DH:(fh + nh) * DH]        # [1024, nh*60]
        blk = blk.reshape(KT, 128, nh * DH).transpose(1, 0, 2)
        wv_chunks.append(
            np.ascontiguousarray(blk.reshape(128, KT * nh * DH)).astype(bf16)
        )

    wout = np.zeros((NSLOT, SLOT, DIM), np.float32)
    wout[:HEADS, 1:DH + 1, :] = w_out.reshape(HEADS, DH, DIM)
    wout[NSLOT - 1, 0, :] = b_out  # bias row at inner index 1088
    # partition-major groups of 3 k-tiles: [3, 128, 3*1020]
    wout = np.ascontiguousarray(
        wout.reshape(3, 3, 128, DIM).transpose(0, 2, 1, 3)
        .reshape(3, 128, 3 * DIM)
    ).astype(bf16)

    # permutation matrix: out[m] = q[partner(m)] (rotate-half pair swap)
    perm = np.zeros((128, 128), np.float32)
    for m in range(128):
        a = m % SLOT
        if a < DH:
            pos = a % D3
            partner = (m // SLOT) * SLOT + (a // D3) * D3 + (
                pos + 10 if pos < 10 else pos - 10
            )
            perm[partner, m] = 1.0
    perm = perm.astype(bf16)

    # rotary tables per core: [128, 512] two identical 64-row head slots
    inv_freq = 1.0 / (10000.0 ** (np.arange(0, D3, 2, dtype=np.float32) / D3))  # [10]
    j = np.arange(SLOT)
    axis_of = np.clip(j // D3, 0, 2)
    jj = (j % D3) % 10
    sign = np.where((j % D3) < 10, -1.0, 1.0).astype(np.float32)
    valid = (j < DH).astype(np.float32)

    in_maps = []
    outs_meta = []
    for c in range(8):
        g, r = c // 4, c % 4
        rows = slice(r * NL, (r + 1) * NL)
        x_loc = np.ascontiguousarray(x[g, rows, :]).astype(bf16)
        t_axis = coords[g, rows, :]  # [NL, 3]
        f = (t_axis[:, axis_of] / MIN_FREQ) * inv_freq[jj][None, :]  # [NL, 64]
        cos_t = (np.cos(f) * valid[None, :]).T.astype(np.float32)  # [64, NL]
        sin_t = (np.sin(f) * (sign * valid)[None, :]).T.astype(np.float32)
        cos_full = np.concatenate([cos_t, cos_t], axis=0).astype(bf16)  # [128, NL]
        sin_full = np.concatenate([sin_t, sin_t], axis=0).astype(bf16)
        im = {
            "x": x_loc,
            "wqk": wqk,
            "wout": wout,
            "cos_t": cos_full,
            "sin_t": sin_full,
            "perm": perm,
        }
        for jj2 in range(4):
            im[f"wv{jj2}"] = wv_chunks[jj2]
        in_maps.append(im)
        outs_meta.append((g, rows))
    return in_maps, outs_meta


def kernel(x, coords, w_qkv, w_out, b_out, _trace=False, _dbg=None):
    from concourse import bass_utils

    in_maps, outs_meta = _host_prep(x, coords, w_qkv, w_out, b_out)
    if "nc" not in _nc_cache:
        _nc_cache["nc"] = _build_nc()
    nc = _nc_cache["nc"]
    last_err = None
    for _attempt in range(3):
        try:
            res = bass_utils.run_bass_kernel_spmd(
                nc, in_maps, core_ids=list(range(8)), trace=_trace
            )
            break
        except Exception as e:  # transient axon worker failures
            last_err = e
            import time as _time
            _time.sleep(2.0)
    else:
        raise last_err
    out = np.empty((B, N, DIM), np.float32)
    for c, (g, rows) in enumerate(outs_meta):
        out[g, rows, :] = res.results[c]["out"]
    if _trace:
        kernel.last_exec_time_ns = res.exec_time_ns
    return out


# revision 21
# speedup vs baseline: 1.2070x; 1.0682x over previous
"""Self-contained Trainium2 Bass kernel for 3D-RoPE multi-head attention.

Problem: x[2,2048,1020] -> qkv proj (17 heads x 60) -> 3D rotary on q,k ->
softmax attention -> out proj + bias.

Strategy: sequence-parallel across 8 NeuronCores (2 batch groups x 4 ranks,
512 rows each). Each core projects its own rows, RoPEs q/k locally, then
AllGathers rotated K and V (merged into 4 chunked collectives fired as
early as possible, in head-pair-aligned chunks) within its 4-core group,
computes attention for its local queries against the full 2048-key
sequence as each chunk lands, and projects the output rows. Matmuls run
in bf16 (f32 PSUM accumulation); softmax skips max-subtraction (logits
are ~N(0,1)); exp runs exclusively on the Scalar engine; PSUM->SBUF
copies in the projection phase run on Scalar (Vector does rope), DMAs are
batched and spread across the sync/vector/gpsimd queues.
"""

import sys

if "/opt/trn_rl_repo" not in sys.path:
    sys.path.insert(0, "/opt/trn_rl_repo")

import numpy as np
import ml_dtypes

HEADS = 17
DH = 60
D3 = 20
MIN_FREQ = 1.0 / 64.0
B, N, DIM = 2, 2048, 1020
NL = 512          # local rows per core
DIMP = 1024       # padded contraction dim (8 k-tiles)
SLOT = 64         # padded per-head column slot
NSLOT = 18        # 17 heads + 1 pad slot
MQK = NSLOT * SLOT  # 1152
NPAIR = 9         # head pairs (last pair has only head A)
KT = 8            # contraction k-tiles (1024/128)
RG = [[0, 1, 2, 3], [4, 5, 6, 7]]

# chunking: 4 collective chunks, pair-aligned
PAIRS_OF = [[0], [1, 2], [3, 4, 5], [6, 7, 8]]
CHUNK_OF_PAIR = [0, 1, 1, 2, 2, 2, 3, 3, 3]
FIRST_HEAD = [0, 2, 6, 12]
NHEADS_OF = [2, 4, 6, 5]
VCOLS_OF = [2 * 61, 4 * 61, 6 * 61, 5 * 61]   # 122,244,366,305
# k rows per pair within a chunk (pair 8 only has head A -> 64 rows)
KROWS_OF_PAIR = [128, 128, 128, 128, 128, 128, 128, 128, 64]


def _chunk_layout(j):
    """k-part offsets (elements) per pair and total sizes for chunk j."""
    offs = {}
    off = 0
    for p in PAIRS_OF[j]:
        offs[p] = off
        off += KROWS_OF_PAIR[p] * NL
    koff_end = off
    vcols = VCOLS_OF[j]
    total = koff_end + 4 * 128 * vcols
    return offs, koff_end, vcols, total


_nc_cache = {}


def _build_nc():
    from concourse import bass, tile, bacc
    import concourse.mybir as mybir
    from concourse.masks import make_identity

    BF = mybir.dt.bfloat16
    F32 = mybir.dt.float32
    AF = mybir.ActivationFunctionType
    ALU = mybir.AluOpType

    nc = bacc.Bacc("TRN2", target_bir_lowering=False, debug=False, num_devices=8)

    # weight tensors are host-prepped in partition-major layout so every
    # DMA line is fully contiguous
    x_ext = nc.declare_dram_parameter("x", [NL, DIM], BF, isOutput=False)
    wqk_ext = nc.declare_dram_parameter("wqk", [2 * NPAIR, 128, KT * 128], BF,
                                        isOutput=False)
    wv_exts = [
        nc.declare_dram_parameter(f"wv{j}", [128, KT * NHEADS_OF[j] * DH], BF,
                                  isOutput=False)
        for j in range(4)
    ]
    wout_ext = nc.declare_dram_parameter("wout", [3, 128, 3 * DIM], BF,
                                         isOutput=False)
    cos_ext = nc.declare_dram_parameter("cos_t", [128, NL], BF, isOutput=False)
    sin_ext = nc.declare_dram_parameter("sin_t", [128, NL], BF, isOutput=False)
    perm_ext = nc.declare_dram_parameter("perm", [128, 128], BF, isOutput=False)
    out_ext = nc.declare_dram_parameter("out", [NL, DIM], F32, isOutput=True)

    SCALE = float(DH) ** -0.5

    with tile.TileContext(nc) as tc:
        with (
            tc.tile_pool(name="per", bufs=1) as per,
            tc.tile_pool(name="wrk", bufs=2) as wrk,
            tc.tile_pool(name="gat", bufs=2) as gat,
            tc.tile_pool(name="expp", bufs=4) as expp,
            tc.tile_pool(name="dram", bufs=1, space="DRAM") as dram,
        ):
            # ---------- persistent SBUF loads (x rows first!) ----------
            x_sb = []
            for mt in range(4):
                xt = per.tile([128, DIM], BF, name=f"xrow{mt}", tag=f"xrow{mt}")
                nc.sync.dma_start(out=xt[:], in_=x_ext[mt * 128:(mt + 1) * 128, :])
                x_sb.append(xt)
            cos_sb = per.tile([128, NL], BF, name="cos", tag="cos")
            nc.sync.dma_start(out=cos_sb[:], in_=cos_ext[:])
            sin_sb = per.tile([128, NL], BF, name="sin", tag="sin")
            nc.sync.dma_start(out=sin_sb[:], in_=sin_ext[:])
            perm_sb = per.tile([128, 128], BF, name="perm", tag="perm")
            nc.sync.dma_start(out=perm_sb[:], in_=perm_ext[:])

            # DRAM bounce buffers for the merged k+v collectives
            kv_loc, kv_gat = [], []
            for j in range(4):
                _, _, _, Lj = _chunk_layout(j)
                kv_loc.append(dram.tile([Lj], BF, name=f"kvl{j}", tag=f"kvl{j}"))
                kv_gat.append(dram.tile([4, Lj], BF, name=f"kvg{j}", tag=f"kvg{j}"))

            # identity for PE transposes (gpsimd) before anything else queues
            ident = per.tile([128, 128], BF, name="ident", tag="ident")
            make_identity(nc, ident[:])

            # weight prefetch (scalar queue; scalar is idle until attention)
            wqkm = [None] * (2 * NPAIR)
            for m in list(range(NPAIR, 2 * NPAIR)) + list(range(NPAIR)):
                t = per.tile([128, KT * 128], BF, name=f"wqkm{m}", tag=f"wqkm{m}")
                nc.scalar.dma_start(out=t[:], in_=wqk_ext[m])
                wqkm[m] = t
            # v weights, one contiguous DMA per chunk (sync queue, after x)
            wv_sb = []
            for j in range(4):
                vc = NHEADS_OF[j] * DH
                t = per.tile([128, KT, vc], BF, name=f"wvs{j}", tag=f"wvs{j}")
                nc.sync.dma_start(
                    out=t[:],
                    in_=wv_exts[j].rearrange("p (k c) -> p k c", k=KT),
                )
                wv_sb.append(t)

            # ---------- phase 1: x -> xT (bf16) ----------
            xT_sb = []
            for k in range(KT):
                t = per.tile([128, NL], BF, name=f"xT{k}", tag=f"xT{k}")
                xT_sb.append(t)
            nc.vector.memset(xT_sb[KT - 1][:], 0.0)

            rotq_sb = [
                per.tile([128, NL], BF, name=f"rotq{m}", tag=f"rotq{m}")
                for m in range(NPAIR)
            ]

            with tc.tile_pool(name="psP", bufs=2, space="PSUM") as psP:
                for mt in range(4):
                    for k in range(KT):
                        kk = min(128, DIM - k * 128)  # 124 on last tile
                        pt = psP.tile([128, 128], BF, name="tp", tag="tp")
                        nc.tensor.transpose(
                            pt[0:kk, :], x_sb[mt][:, k * 128:k * 128 + kk], ident[:]
                        )
                        nc.vector.tensor_copy(
                            xT_sb[k][0:kk, mt * 128:(mt + 1) * 128], pt[0:kk, :]
                        )

                def qk_tile(m, dest):
                    # m: M-tile index into [0, 18): 0-8 q slots, 9-17 k slots
                    pqk = psP.tile([128, NL], F32, name="qk", tag="qk", bufs=4)
                    for k in range(KT):
                        nc.tensor.matmul(
                            pqk[:],
                            lhsT=wqkm[m][:, k * 128:(k + 1) * 128],
                            rhs=xT_sb[k][:],
                            start=(k == 0),
                            stop=(k == KT - 1),
                        )
                    qkbf = wrk.tile([128, NL], BF, name="qkbf", tag="qkbf")
                    nc.vector.tensor_copy(qkbf[:], pqk[:])
                    psw = psP.tile([128, NL], F32, name="sw", tag="sw")
                    nc.tensor.matmul(psw[:], lhsT=perm_sb[:], rhs=qkbf[:])
                    t1 = wrk.tile([128, NL], BF, name="t1", tag="t1")
                    nc.gpsimd.tensor_tensor(t1[:], qkbf[:], cos_sb[:], ALU.mult)
                    t2 = wrk.tile([128, NL], BF, name="t2", tag="t2")
                    nc.vector.tensor_tensor(t2[:], psw[:], sin_sb[:], ALU.mult)
                    nc.gpsimd.tensor_tensor(dest[:], t1[:], t2[:], ALU.add)

                def k_pair(p):
                    j = CHUNK_OF_PAIR[p]
                    offs, _, _, _ = _chunk_layout(j)
                    rows = KROWS_OF_PAIR[p]
                    dest = wrk.tile([128, NL], BF, name="rotk", tag="rotk")
                    qk_tile(NPAIR + p, dest)
                    nc.sync.dma_start(
                        out=kv_loc[j][offs[p]:offs[p] + rows * NL]
                        .rearrange("(p n) -> p n", p=rows),
                        in_=dest[0:rows, :],
                    )

                def v_chunk(j):
                    offs, koff_end, vcols, _ = _chunk_layout(j)
                    nh = NHEADS_OF[j]
                    nc_cols = nh * DH
                    for mt in range(4):
                        vxt = wrk.tile([128, 366], BF, name="vx", tag="vx")
                        ones_ap = vxt[:, 0:vcols].rearrange(
                            "p (h c) -> p h c", c=61
                        )[:, :, 0:1]
                        nc.gpsimd.memset(ones_ap, 1.0)
                        pv = psP.tile([128, 384], F32, name="pv", tag="qk", bufs=4)
                        for k in range(KT):
                            nc.tensor.matmul(
                                pv[:, 0:nc_cols],
                                lhsT=xT_sb[k][:, mt * 128:(mt + 1) * 128],
                                rhs=wv_sb[j][:, k, :],
                                start=(k == 0),
                                stop=(k == KT - 1),
                            )
                        src = pv[:, 0:nc_cols].rearrange("p (h d) -> p h d", d=DH)
                        dst = vxt[:, 0:vcols].rearrange(
                            "p (h c) -> p h c", c=61
                        )[:, :, 1:61]
                        nc.vector.tensor_copy(dst, src)
                        nc.sync.dma_start(
                            out=kv_loc[j][koff_end + mt * 128 * vcols:
                                          koff_end + (mt + 1) * 128 * vcols]
                            .rearrange("(p c) -> p c", p=128),
                            in_=vxt[:, 0:vcols],
                        )

                def ag_fire(j):
                    nc.gpsimd.collective_compute(
                        "AllGather", ALU.bypass,
                        ins=[kv_loc[j].opt()], outs=[kv_gat[j].opt()],
                        replica_groups=RG,
                    )

                # k+v projections in chunk order, firing AGs as ready;
                # q projections interleaved between chunks so early pairs'
                # rotq are ready as soon as their AG lands
                Q_AFTER = [[0], [1, 2], [3, 4, 5], [6, 7, 8]]
                for j in range(4):
                    for p in PAIRS_OF[j]:
                        k_pair(p)
                    v_chunk(j)
                    ag_fire(j)
                    for m in Q_AFTER[j]:
                        qk_tile(m, rotq_sb[m])

            # wout loads (batched, gpsimd queue; needed only at the tail)
            wout_sb = []
            for g in range(3):
                t = per.tile([128, 3, DIM], BF, name=f"wout{g}", tag=f"wout{g}")
                nc.gpsimd.dma_start(
                    out=t[:],
                    in_=wout_ext[g].rearrange("p (t c) -> p t c", t=3),
                )
                wout_sb.append(t)

            # ---------- attention ----------
            aoT = [
                per.tile([128, NL], BF, name=f"aoT{p}", tag=f"aoT{p}")
                for p in range(NPAIR)
            ]
            for p in range(NPAIR):
                nc.gpsimd.memset(aoT[p][:], 0.0)
            # bias row (inner index 1088 = slot 17 row 0 -> tile 8, partition 64)
            nc.vector.memset(aoT[NPAIR - 1][64:65, :], 1.0)

            # gathered tiles (SBUF), batched DMAs per chunk; the ktp/vxg
            # tiles rotate through 2 buffers (chunk j+2 reuses chunk j's)
            ktp_sb = []    # [128, 2, 4, NL]: (slot-row, pair-in-chunk, rank, keypos)
            vxg_sb = []    # [128, 4, 4, vcols]: (keypos-in-mt, rank, mt, vcol)
            ktp8_sb = None
            for j in range(4):
                offs, koff_end, vcols, Lj = _chunk_layout(j)
                n128 = len([p for p in PAIRS_OF[j] if KROWS_OF_PAIR[p] == 128])
                kt_t = gat.tile([128, 3, 4, NL], BF, name=f"ktp{j}", tag="ktp")
                for r in range(4):
                    nc.sync.dma_start(
                        out=kt_t[:, 0:n128, r, :],
                        in_=kv_gat[j][r, 0:n128 * 128 * NL]
                        .rearrange("(pp p n) -> p pp n", pp=n128, p=128),
                    )
                ktp_sb.append(kt_t)
                if j == 3:
                    ktp8_sb = per.tile([64, 4, NL], BF, name="ktp8", tag="ktp8")
                    nc.sync.dma_start(
                        out=ktp8_sb[:],
                        in_=kv_gat[3][:, n128 * 128 * NL:n128 * 128 * NL + 64 * NL]
                        .rearrange("r (p n) -> p r n", p=64),
                    )
                vx_t = gat.tile([128, 4, 4, 366], BF, name=f"vxg{j}", tag="vxg")
                for r in range(4):
                    nc.sync.dma_start(
                        out=vx_t[:, r, :, 0:vcols],
                        in_=kv_gat[j][r, koff_end:koff_end + 4 * 128 * vcols]
                        .rearrange("(mt p c) -> p mt c", mt=4, p=128),
                    )
                vxg_sb.append(vx_t)

            with (
                tc.tile_pool(name="psD", bufs=3, space="PSUM") as psD,
                tc.tile_pool(name="psAV", bufs=2, space="PSUM") as psAV,
            ):
                def pair_attention(p):
                    j = CHUNK_OF_PAIR[p]
                    two = p < NPAIR - 1
                    pp = PAIRS_OF[j].index(p)
                    lA = (2 * p - FIRST_HEAD[j]) * 61
                    lB = lA + 61
                    av = psAV.tile([128, NL], F32, name="av", tag="av")
                    for c in range(16):
                        r, i = c // 4, c % 4
                        if p == NPAIR - 1:
                            kA = ktp8_sb[0:DH, r, i * 128:(i + 1) * 128]
                            kB = None
                        else:
                            kA = ktp_sb[j][0:DH, pp, r, i * 128:(i + 1) * 128]
                            kB = ktp_sb[j][64:64 + DH, pp, r, i * 128:(i + 1) * 128]
                        dots = psD.tile([128, 1024], F32, name="dots", tag="dots")
                        nc.tensor.matmul(dots[:, 0:NL], lhsT=kA,
                                         rhs=rotq_sb[p][0:DH, :])
                        if two:
                            nc.tensor.matmul(dots[:, NL:2 * NL], lhsT=kB,
                                             rhs=rotq_sb[p][64:64 + DH, :])
                        et = expp.tile([128, 1024], BF, name="et", tag="expT")
                        width = 1024 if two else NL
                        nc.scalar.activation(et[:, 0:width], dots[:, 0:width],
                                             AF.Exp, scale=SCALE)
                        nc.tensor.matmul(
                            av[0:61, :],
                            lhsT=vxg_sb[j][:, r, i, lA:lA + 61],
                            rhs=et[:, 0:NL],
                            start=(c == 0), stop=(c == 15),
                        )
                        if two:
                            nc.tensor.matmul(
                                av[64:125, :],
                                lhsT=vxg_sb[j][:, r, i, lB:lB + 61],
                                rhs=et[:, NL:2 * NL],
                                start=(c == 0), stop=(c == 15),
                            )
                    # epilogue: divide by the ones-row denominator
                    rcA = wrk.tile([1, NL], F32, name="rc", tag="rc")
                    rcA_s = wrk.tile([1, NL], F32, name="rcs", tag="rcs")
                    nc.vector.tensor_copy(rcA_s[:], av[0:1, :])
                    nc.vector.reciprocal_approx_fast(rcA[:], rcA_s[:])
                    bc = wrk.tile([128, NL], F32, name="bc", tag="bc")
                    nc.gpsimd.partition_broadcast(bc[0:61, :], rcA[:])
                    nc.vector.tensor_tensor(
                        aoT[p][0:61, :], av[0:61, :], bc[0:61, :], ALU.mult
                    )
                    if two:
                        rcB = wrk.tile([1, NL], F32, name="rc", tag="rc")
                        rcB_s = wrk.tile([1, NL], F32, name="rcs", tag="rcs")
                        nc.vector.tensor_copy(rcB_s[:], av[64:65, :])
                        nc.vector.reciprocal_approx_fast(rcB[:], rcB_s[:])
                        bc2 = wrk.tile([128, NL], F32, name="bc2", tag="bc2")
                        nc.gpsimd.partition_broadcast(bc2[0:61, :], rcB[:])
                        nc.vector.tensor_tensor(
                            aoT[p][64:125, :], av[64:125, :], bc2[0:61, :],
                            ALU.mult,
                        )

                for p in range(NPAIR):
                    pair_attention(p)

            # ---------- output projection ----------
            with tc.tile_pool(name="psO", bufs=2, space="PSUM") as psO:
                for mt in range(4):
                    for (n0, n1) in ((0, 510), (510, 1020)):
                        po = psO.tile([128, 510], F32, name="po", tag="o")
                        for kt in range(NPAIR):
                            nc.tensor.matmul(
                                po[:],
                                lhsT=aoT[kt][:, mt * 128:(mt + 1) * 128],
                                rhs=wout_sb[kt // 3][:, kt % 3, n0:n1],
                                start=(kt == 0),
                                stop=(kt == NPAIR - 1),
                            )
                        ot = wrk.tile([128, 510], F32, name="ot", tag="ot")
                        nc.vector.tensor_copy(ot[:], po[:])
                        nc.sync.dma_start(
                            out=out_ext[mt * 128:(mt + 1) * 128, n0:n1], in_=ot[:]
                        )

    nc.finalize()
    return nc


def _host_prep(x, coords, w_qkv, w_out, b_out):
    bf16 = ml_dtypes.bfloat16
    x = np.asarray(x, np.float32)
    coords = np.asarray(coords, np.float32)
    w_qkv = np.asarray(w_qkv, np.float32)
    w_out = np.asarray(w_out, np.float32)
    b_out = np.asarray(b_out, np.float32)

    # weights: q/k into 64-wide head slots, [1024, 2*1152]
    wqk = np.zeros((DIMP, 2 * MQK), np.float32)
    wq = w_qkv[:, 0:DIM].reshape(DIM, HEADS, DH)
    wk = w_qkv[:, DIM:2 * DIM].reshape(DIM, HEADS, DH)
    t = np.zeros((DIM, NSLOT, SLOT), np.float32)
    t[:, :HEADS, :DH] = wq
    wqk[:DIM, 0:MQK] = t.reshape(DIM, MQK)
    t[:] = 0.0
    t[:, :HEADS, :DH] = wk
    wqk[:DIM, MQK:2 * MQK] = t.reshape(DIM, MQK)
    # M-tile-major then partition-major: [18, 128, KT*128]
    # arr[m, p, k*128+c] = wqk[k*128+p, m*128+c]
    wqk = np.ascontiguousarray(
        wqk.reshape(KT, 128, 2 * NPAIR, 128).transpose(2, 1, 0, 3)
        .reshape(2 * NPAIR, 128, KT * 128)
    ).astype(bf16)

    # v weights, per-chunk partition-major [128, KT*nh*60]
    wv_full = np.zeros((DIMP, DIM), np.float32)
    wv_full[:DIM, :] = w_qkv[:, 2 * DIM:3 * DIM]
    wv_chunks = []
    for j in range(4):
        fh, nh = FIRST_HEAD[j], NHEADS_OF[j]
        blk = wv_full[:, fh * DH:(fh + nh) * DH]        # [1024, nh*60]
        blk = blk.reshape(KT, 128, nh * DH).transpose(1, 0, 2)
        wv_chunks.append(
            np.ascontiguousarray(blk.reshape(128, KT * nh * DH)).astype(bf16)
        )

    wout = np.zeros((NSLOT, SLOT, DIM), np.float32)
    wout[:HEADS, 1:DH + 1, :] = w_out.reshape(HEADS, DH, DIM)
    wout[NSLOT - 1, 0, :] = b_out  # bias row at inner index 1088
    # partition-major groups of 3 k-tiles: [3, 128, 3*1020]
    wout = np.ascontiguousarray(
        wout.reshape(3, 3, 128, DIM).transpose(0, 2, 1, 3)
        .reshape(3, 128, 3 * DIM)
    ).astype(bf16)

    # permutation matrix: out[m] = q[partner(m)] (rotate-half pair swap)
    perm = np.zeros((128, 128), np.float32)
    for m in range(128):
        a = m % SLOT
        if a < DH:
            pos = a % D3
            partner = (m // SLOT) * SLOT + (a // D3) * D3 + (
                pos + 10 if pos < 10 else pos - 10
            )
            perm[partner, m] = 1.0
    perm = perm.astype(bf16)

    # rotary tables per core: [128, 512] two identical 64-row head slots
    inv_freq = 1.0 / (10000.0 ** (np.arange(0, D3, 2, dtype=np.float32) / D3))  # [10]
    j = np.arange(SLOT)
    axis_of = np.clip(j // D3, 0, 2)
    jj = (j % D3) % 10
    sign = np.where((j % D3) < 10, -1.0, 1.0).astype(np.float32)
    valid = (j < DH).astype(np.float32)

    in_maps = []
    outs_meta = []
    for c in range(8):
        g, r = c // 4, c % 4
        rows = slice(r * NL, (r + 1) * NL)
        x_loc = np.ascontiguousarray(x[g, rows, :]).astype(bf16)
        t_axis = coords[g, rows, :]  # [NL, 3]
        f = (t_axis[:, axis_of] / MIN_FREQ) * inv_freq[jj][None, :]  # [NL, 64]
        cos_t = (np.cos(f) * valid[None, :]).T.astype(np.float32)  # [64, NL]
        sin_t = (np.sin(f) * (sign * valid)[None, :]).T.astype(np.float32)
        cos_full = np.concatenate([cos_t, cos_t], axis=0).astype(bf16)  # [128, NL]
        sin_full = np.concatenate([sin_t, sin_t], axis=0).astype(bf16)
        im = {
            "x": x_loc,
            "wqk": wqk,
            "wout": wout,
            "cos_t": cos_full,
            "sin_t": sin_full,
            "perm": perm,
        }
        for jj2 in range(4):
            im[f"wv{jj2}"] = wv_chunks[jj2]
        in_maps.append(im)
        outs_meta.append((g, rows))
    return in_maps, outs_meta


def kernel(x, coords, w_qkv, w_out, b_out, _trace=False, _dbg=None):
    from concourse import bass_utils

    in_maps, outs_meta = _host_prep(x, coords, w_qkv, w_out, b_out)
    if "nc" not in _nc_cache:
        _nc_cache["nc"] = _build_nc()
    nc = _nc_cache["nc"]
    last_err = None
    for _attempt in range(3):
        try:
            res = bass_utils.run_bass_kernel_spmd(
                nc, in_maps, core_ids=list(range(8)), trace=_trace
            )
            break
        except Exception as e:  # transient axon worker failures
            last_err = e
            import time as _time
            _time.sleep(2.0)
    else:
        raise last_err
    out = np.empty((B, N, DIM), np.float32)
    for c, (g, rows) in enumerate(outs_meta):
        out[g, rows, :] = res.results[c]["out"]
    if _trace:
        kernel.last_exec_time_ns = res.exec_time_ns
    return out
